# revision 61
# baseline (speedup 1.0000x reference)
"""Trainium2 Bass kernel for nn_AutoReg (4-layer dense transformer, teacher forcing).

Sharding across 8 NeuronCores: data-parallel over batch (B=4 -> 4 core pairs),
sequence-split within each pair (causal-balanced interleaved row blocks).
Per-layer K/V are exchanged with one 8-rank AllGather; attention is split into
a LOCAL pass (own K/V straight from SBUF, runs concurrently with the
AllGather) and a REMOTE pass (partner K/V, read back with partition_id-based
dynamic DMA offsets), so the SPMD program is identical on every core and the
collective hides behind real compute.

Math: bf16 matmul inputs with fp32 PSUM accumulation; LayerNorm, softmax and
the residual stream in fp32.  Scores are computed transposed (sT[rk, rq]) so
the softmax denominator comes out of the AV matmul via an appended ones
column; masking is a binary multiply on the exp output; normalization uses a
fast approximate reciprocal plus a K=1 broadcast matmul.
"""

import numpy as np
import ml_dtypes

import concourse.bass as bass
import concourse.bacc as bacc
import concourse.mybir as mybir
import concourse.tile as tile
from concourse.bass import ds
from concourse.bass_utils import run_bass_kernel_spmd
from concourse.masks import make_identity

# Model dims (hardcoded per the problem spec)
L, B, S, D, H, F = 4, 4, 1024, 1024, 16, 4096
V1, V2, OUT = 32, 16, 50
HD = D // H            # 64
NCORES = 8
RLOC = 512             # local rows per core
NRB = RLOC // 128      # 4 local row blocks
NC_ = D // 128         # 8 D-chunks
NFO = F // 128         # 32 F-chunks
SCALE = 1.0 / np.sqrt(HD)

# global row-block assignment per parity (causal-balanced):
# parity 0 (even cores) own blocks [0,1,6,7]; parity 1 own [2,3,4,5]
BLOCKS = {0: [0, 1, 6, 7], 1: [2, 3, 4, 5]}

BF = mybir.dt.bfloat16
F32 = mybir.dt.float32

KSEG = D * RLOC              # elems: kT region of one core's kv block
VSEG = RLOC * D              # elems: v region
SEG = KSEG + VSEG            # elems per rank in the AllGather


def _build_program():
    nc = bacc.Bacc("TRN2", target_bir_lowering=False)

    # ---- DRAM parameters (per-core inputs) ----
    eat_in = nc.declare_dram_parameter("eat", [64, RLOC], BF, isOutput=False)
    wa_in = nc.declare_dram_parameter("wa", [64, D], BF, isOutput=False)
    pos_in = nc.declare_dram_parameter("pos", [RLOC, D], F32, isOutput=False)
    masks_in = nc.declare_dram_parameter("masks", [128, 8, RLOC], BF, isOutput=False)
    wq_in = nc.declare_dram_parameter("wq", [L * D, D], BF, isOutput=False)
    wk_in = nc.declare_dram_parameter("wk", [L * D, D], BF, isOutput=False)
    wv_in = nc.declare_dram_parameter("wv", [L * D, D], BF, isOutput=False)
    wo_in = nc.declare_dram_parameter("wo", [L * D, D], BF, isOutput=False)
    w1_in = nc.declare_dram_parameter("w1", [L * D, F], BF, isOutput=False)
    w2_in = nc.declare_dram_parameter("w2", [L * F, D], BF, isOutput=False)
    b1_in = nc.declare_dram_parameter("b1", [L * F], F32, isOutput=False)
    b2_in = nc.declare_dram_parameter("b2", [L * D], F32, isOutput=False)
    ln1g_in = nc.declare_dram_parameter("ln1g", [L * D], F32, isOutput=False)
    ln1b_in = nc.declare_dram_parameter("ln1b", [L * D], F32, isOutput=False)
    ln2g_in = nc.declare_dram_parameter("ln2g", [L * D], F32, isOutput=False)
    ln2b_in = nc.declare_dram_parameter("ln2b", [L * D], F32, isOutput=False)
    lnfg_in = nc.declare_dram_parameter("lnfg", [D], F32, isOutput=False)
    lnfb_in = nc.declare_dram_parameter("lnfb", [D], F32, isOutput=False)
    wd_in = nc.declare_dram_parameter("wd", [D, OUT], BF, isOutput=False)
    bd_in = nc.declare_dram_parameter("bd", [OUT], F32, isOutput=False)
    out_p = nc.declare_dram_parameter("out", [RLOC, OUT], F32, isOutput=True)

    def bcast_ap(src_ap, p=128):
        """Partition-broadcast view of a 1-D DRAM AP."""
        return bass.AP(tensor=src_ap.tensor, offset=src_ap.offset,
                       ap=[[0, p]] + [list(x) for x in src_ap.ap])

    AF = mybir.ActivationFunctionType
    ALU = mybir.AluOpType

    with tile.TileContext(nc) as tc:
        with tc.tile_pool(name="res", bufs=1) as res, \
             tc.tile_pool(name="wbig", bufs=2) as wbig, \
             tc.tile_pool(name="yt", bufs=1) as ytp, \
             tc.tile_pool(name="xt", bufs=2) as xtp, \
             tc.tile_pool(name="expp", bufs=3) as expp, \
             tc.tile_pool(name="xc", bufs=2) as xcp, \
             tc.tile_pool(name="prm", bufs=2) as prm, \
             tc.tile_pool(name="sm", bufs=4) as sm, \
             tc.tile_pool(name="dr", bufs=1, space="DRAM") as dr, \
             tc.tile_pool(name="ps_big", bufs=2, space="PSUM") as ps_big, \
             tc.tile_pool(name="ps_s", bufs=2, space="PSUM") as ps_s, \
             tc.tile_pool(name="ps_av", bufs=2, space="PSUM") as ps_av:

            # ---- resident tiles ----
            h_sb = res.tile([128, NRB, D], F32)            # residual stream
            kst = res.tile([128, NC_, RLOC], BF)           # own K^T
            kT_rem = res.tile([128, NC_, RLOC], BF)        # partner K^T
            v_loc = res.tile([128, NRB, H, HD + 1], BF)    # own V + ones col
            v_rem = res.tile([128, NRB, H, HD + 1], BF)    # partner V + ones col
            qT_sb = res.tile([128, NC_, RLOC], BF)
            oT_sb = res.tile([128, NC_, RLOC], BF)
            p1_sb = res.tile([HD + 1, H, RLOC], BF)        # pass-1 partial (o|sum)
            eat_sb = res.tile([64, RLOC], BF)
            wa_sb = res.tile([64, D], BF)
            ident = res.tile([128, 128], F32)
            ones64 = res.tile([1, 64], BF)
            wd_sb = res.tile([128, NC_, OUT], BF)
            bd_bc = res.tile([128, OUT], F32)
            eps_sb = res.tile([128, 1], F32)

            make_identity(nc, ident)
            nc.vector.memset(eps_sb, 1e-6)
            nc.vector.memset(ones64, 1.0)
            nc.vector.memset(v_loc[:, :, :, HD:HD + 1], 1.0)
            nc.vector.memset(v_rem[:, :, :, HD:HD + 1], 1.0)
            nc.sync.dma_start(eat_sb, eat_in[:, :])
            nc.sync.dma_start(wa_sb, wa_in[:, :])
            nc.sync.dma_start(wd_sb, wd_in.rearrange("(c p) n -> p c n", p=128))
            nc.sync.dma_start(bd_bc, bcast_ap(bd_in[:]))

            # dynamic base: partner's segment offset in the pair AllGather output
            pid = nc.sync.partition_id()
            par = pid - (pid // 2) * 2
            rem_base = (1 - par) * SEG

            # ---- embedding: h = EaT^T @ Wa + pos ----
            pos_sb = wbig.tile([128, NRB, D], F32, tag="w2mb")
            nc.sync.dma_start(pos_sb, pos_in.rearrange("(rb p) d -> p rb d", p=128))
            for rb in range(NRB):
                for o2 in range(2):
                    ps = ps_big.tile([128, 512], F32, tag="big")
                    nc.tensor.matmul(ps, eat_sb[:, 128 * rb:128 * (rb + 1)],
                                     wa_sb[:, 512 * o2:512 * (o2 + 1)],
                                     start=True, stop=True)
                    nc.vector.tensor_add(h_sb[:, rb, 512 * o2:512 * (o2 + 1)],
                                         pos_sb[:, rb, 512 * o2:512 * (o2 + 1)], ps)

            # warm up the ACT exp/ln table set
            warm = sm.tile([128, 1], F32, tag="s1")
            nc.vector.memset(warm, 1.0)
            nc.scalar.activation(warm, warm, AF.Ln, bias=eps_sb, scale=1.0)
            nc.scalar.activation(warm, warm, AF.Exp, bias=0.0, scale=-0.5)

            def layernorm_to_xT(g_src, b_src, xT):
                """LN(h) with affine (g,b), transposed into xT [128, NC_, RLOC] bf16."""
                g_sb = prm.tile([128, NC_], F32, tag="lng")
                b_sb = prm.tile([128, NC_], F32, tag="lnb")
                nc.sync.dma_start(g_sb, g_src.rearrange("(c p) -> p c", p=128))
                nc.sync.dma_start(b_sb, b_src.rearrange("(c p) -> p c", p=128))
                for rb in range(NRB):
                    stats = sm.tile([128, 2, 6], F32, tag="st")
                    nc.vector.bn_stats(stats[:, 0, :], h_sb[:, rb, 0:512])
                    nc.vector.bn_stats(stats[:, 1, :], h_sb[:, rb, 512:1024])
                    mv = sm.tile([128, 2], F32, tag="mv")
                    nc.vector.bn_aggr(mv, stats)
                    rstd = sm.tile([128, 1], F32, tag="rstd")
                    nc.scalar.activation(rstd, mv[:, 1:2], AF.Ln, bias=eps_sb, scale=1.0)
                    nc.scalar.activation(rstd, rstd, AF.Exp, bias=0.0, scale=-0.5)
                    xc = xcp.tile([128, D], F32, tag="xc")
                    nc.vector.tensor_scalar(xc, h_sb[:, rb, :], mv[:, 0:1], rstd,
                                            ALU.subtract, ALU.mult)
                    for c in range(NC_):
                        tp = ps_s.tile([128, 128], F32, tag="s")
                        nc.tensor.transpose(tp, xc[:, 128 * c:128 * (c + 1)], ident)
                        nc.vector.tensor_scalar(
                            xT[:, c, 128 * rb:128 * (rb + 1)], tp,
                            g_sb[:, c:c + 1], b_sb[:, c:c + 1], ALU.mult, ALU.add)

            def load_w(src2d, tag="w2mb"):
                w = wbig.tile([128, NC_, src2d.shape[1]], BF, tag=tag)
                nc.sync.dma_start(w, src2d.rearrange("(c p) n -> p c n", p=128))
                return w

            def attn_pass_pair(i, kt, vt, mask_sb, av0, av1, jbase):
                """One attention pass (4 rk blocks) for head pair (2i, 2i+1).

                The two heads' score matmuls contract over disjoint partition
                halves of kT/qT, so the PE runs them concurrently."""
                h0, h1 = 2 * i, 2 * i + 1
                expT0 = expp.tile([128, 4, RLOC], BF, tag="exp")
                expT1 = expp.tile([128, 4, RLOC], BF, tag="exp")
                for jp in range(2):
                    st0 = ps_s.tile([128, 2, RLOC], F32, tag="s")
                    st1 = ps_s.tile([128, 2, RLOC], F32, tag="s")
                    for dj in range(2):
                        j = 2 * jp + dj
                        nc.tensor.matmul(st0[:, dj, :],
                                         kt[0:64, i, 128 * j:128 * (j + 1)],
                                         qT_sb[0:64, i, :], start=True, stop=True)
                        nc.tensor.matmul(st1[:, dj, :],
                                         kt[64:128, i, 128 * j:128 * (j + 1)],
                                         qT_sb[64:128, i, :], start=True, stop=True)
                    msl = mask_sb[:, jbase + 2 * jp:jbase + 2 * jp + 2, :]
                    nc.scalar.activation(expT0[:, 2 * jp:2 * jp + 2, :], st0, AF.Exp)
                    nc.vector.tensor_tensor(
                        expT0[:, 2 * jp:2 * jp + 2, :], expT0[:, 2 * jp:2 * jp + 2, :],
                        msl, ALU.mult)
                    nc.scalar.activation(expT1[:, 2 * jp:2 * jp + 2, :], st1, AF.Exp)
                    nc.vector.tensor_tensor(
                        expT1[:, 2 * jp:2 * jp + 2, :], expT1[:, 2 * jp:2 * jp + 2, :],
                        msl, ALU.mult)
                for j in range(4):
                    nc.tensor.matmul(av0, vt[:, j, h0, :], expT0[:, j, :],
                                     start=(j == 0), stop=(j == 3))
                for j in range(4):
                    nc.tensor.matmul(av1, vt[:, j, h1, :], expT1[:, j, :],
                                     start=(j == 0), stop=(j == 3))

            for l in range(L):
                # per-layer kv exchange bounce buffers (Shared tiles allow one writer)
                kv_in = dr.tile([SEG], BF, tag="kvin", name=f"kv_in_{l}")
                kv_all = dr.tile([2 * SEG], BF, tag="kvall", name=f"kv_all_{l}")

                # ---- LN1 -> xT ----
                xT = xtp.tile([128, NC_, RLOC], BF, tag="xt")
                layernorm_to_xT(ln1g_in[ds(D * l, D)], ln1b_in[ds(D * l, D)], xT)

                # ---- K projection (own rows) -> kst + bounce ----
                wk_sb = load_w(wk_in[D * l:D * (l + 1)])
                for o in range(NC_):
                    ps = ps_big.tile([128, 512], F32, tag="big")
                    for c in range(NC_):
                        nc.tensor.matmul(ps, wk_sb[:, c, 128 * o:128 * (o + 1)],
                                         xT[:, c, :], start=(c == 0), stop=(c == NC_ - 1))
                    nc.scalar.copy(kst[:, o, :], ps)
                nc.sync.dma_start(
                    kv_in[0:KSEG].rearrange("(o p r) -> p o r", o=NC_, p=128), kst)

                # ---- V projection (own rows) -> v_loc + bounce ----
                wv_sb = load_w(wv_in[D * l:D * (l + 1)])
                for rb in range(NRB):
                    for o2 in range(2):
                        ps = ps_big.tile([128, 512], F32, tag="big")
                        for c in range(NC_):
                            nc.tensor.matmul(ps, xT[:, c, 128 * rb:128 * (rb + 1)],
                                             wv_sb[:, c, 512 * o2:512 * (o2 + 1)],
                                             start=(c == 0), stop=(c == NC_ - 1))
                        nc.scalar.copy(
                            v_loc[:, rb, 8 * o2:8 * (o2 + 1), 0:HD],
                            ps.rearrange("p (hh e) -> p hh e", hh=8))
                for rb in range(NRB):
                    nc.sync.dma_start(
                        kv_in[KSEG + rb * 128 * D:KSEG + (rb + 1) * 128 * D].rearrange(
                            "(p hh e) -> p hh e", p=128, hh=H),
                        v_loc[:, rb, :, 0:HD])

                # ---- single AllGather of K,V across all 8 cores ----
                nc.gpsimd.collective_compute(
                    "AllGather", ALU.bypass,
                    replica_groups=[[0, 1], [2, 3], [4, 5], [6, 7]],
                    ins=[kv_in[:]], outs=[kv_all[:]])

                # ---- Q projection (overlaps the AllGather) ----
                wq_sb = load_w(wq_in[D * l:D * (l + 1)])
                for o in range(NC_):
                    ps = ps_big.tile([128, 512], F32, tag="big")
                    for c in range(NC_):
                        nc.tensor.matmul(ps, wq_sb[:, c, 128 * o:128 * (o + 1)],
                                         xT[:, c, :], start=(c == 0), stop=(c == NC_ - 1))
                    nc.scalar.mul(qT_sb[:, o, :], ps, float(SCALE))

                # ---- attention pass 1: own K/V (no AllGather dependency) ----
                mask_sb = ytp.tile([128, 8, RLOC], BF, tag="yt")
                nc.sync.dma_start(mask_sb, masks_in[:, :, :])
                for i in range(H // 2):
                    pa = ps_av.tile([HD + 1, RLOC], F32, tag="av")
                    pb = ps_av.tile([HD + 1, RLOC], F32, tag="av")
                    attn_pass_pair(i, kst, v_loc, mask_sb, pa, pb, 0)
                    nc.vector.tensor_copy(p1_sb[:, 2 * i, :], pa)
                    nc.vector.tensor_copy(p1_sb[:, 2 * i + 1, :], pb)

                # ---- fetch partner K/V from the AllGather ----
                ksrc = kv_all[ds(rem_base, KSEG)].rearrange(
                    "(o p r) -> p o r", o=NC_, p=128)
                nc.sync.dma_start(kT_rem, ksrc)
                for rb in range(NRB):
                    vsrc = kv_all[ds(rem_base + KSEG + rb * 128 * D, 128 * D)].rearrange(
                        "(p hh e) -> p hh e", p=128, hh=H)
                    nc.sync.dma_start(v_rem[:, rb, :, 0:HD], vsrc)

                # ---- attention pass 2: partner K/V, combine, normalize ----
                def finalize_head(h, av2):
                    hp, ho = 64 * (h % 2), h // 2
                    nc.vector.tensor_tensor(av2, av2, p1_sb[:, h, :], ALU.add)
                    nc.vector.tensor_copy(oT_sb[hp:hp + 64, ho, :], av2[0:HD, :])
                    s1 = sm.tile([1, RLOC], F32, tag="s1h", bufs=2)
                    nc.vector.tensor_copy(s1, av2[HD:HD + 1, :])
                    rcf = sm.tile([1, RLOC], F32, tag="rcf", bufs=2)
                    nc.vector.reciprocal_approx_fast(rcf, s1)
                    rc1 = sm.tile([1, RLOC], BF, tag="rc1", bufs=2)
                    nc.vector.tensor_copy(rc1, rcf)
                    bc = ps_big.tile([64, RLOC], F32, tag="big")
                    nc.tensor.matmul(bc, ones64, rc1, start=True, stop=True)
                    bc_sb = sm.tile([128, RLOC], BF, tag="bcsb", bufs=2)
                    nc.vector.tensor_copy(bc_sb[hp:hp + 64, :], bc)
                    nc.vector.tensor_tensor(oT_sb[hp:hp + 64, ho, :],
                                            oT_sb[hp:hp + 64, ho, :],
                                            bc_sb[hp:hp + 64, :], ALU.mult)

                wo_sb = load_w(wo_in[D * l:D * (l + 1)])
                for i in range(H // 2):
                    pa = ps_av.tile([HD + 1, RLOC], F32, tag="av")
                    pb = ps_av.tile([HD + 1, RLOC], F32, tag="av")
                    attn_pass_pair(i, kT_rem, v_rem, mask_sb, pa, pb, 4)
                    finalize_head(2 * i, pa)
                    finalize_head(2 * i + 1, pb)

                # ---- output projection + residual ----
                for rb in range(NRB):
                    for o2 in range(2):
                        ps = ps_big.tile([128, 512], F32, tag="big")
                        for c in range(NC_):
                            nc.tensor.matmul(ps, oT_sb[:, c, 128 * rb:128 * (rb + 1)],
                                             wo_sb[:, c, 512 * o2:512 * (o2 + 1)],
                                             start=(c == 0), stop=(c == NC_ - 1))
                        hsl = h_sb[:, rb, 512 * o2:512 * (o2 + 1)]
                        nc.vector.tensor_add(hsl, hsl, ps)

                # ---- LN2 -> xT2 ----
                xT2 = xtp.tile([128, NC_, RLOC], BF, tag="xt")
                layernorm_to_xT(ln2g_in[ds(D * l, D)], ln2b_in[ds(D * l, D)], xT2)

                # ---- FFN1: yT = relu(w1^T x + b1) ----
                b1_sb = prm.tile([128, NFO], F32, tag="b1")
                nc.sync.dma_start(b1_sb, b1_in[ds(F * l, F)].rearrange("(o p) -> p o", p=128))
                yT = ytp.tile([128, NFO, RLOC], BF, tag="yt")
                for phi in range(4):
                    w1_sb = load_w(w1_in[D * l:D * (l + 1), 1024 * phi:1024 * (phi + 1)])
                    for fo in range(8):
                        fg = 8 * phi + fo
                        ps = ps_big.tile([128, 512], F32, tag="big")
                        for c in range(NC_):
                            nc.tensor.matmul(ps, w1_sb[:, c, 128 * fo:128 * (fo + 1)],
                                             xT2[:, c, :], start=(c == 0), stop=(c == NC_ - 1))
                        nc.scalar.activation(yT[:, fg, :], ps, AF.Relu,
                                             bias=b1_sb[:, fg:fg + 1], scale=1.0)

                # ---- FFN2: h += yT^T @ w2 (+ b2) ----
                for phi in range(4):
                    w2_sb = load_w(w2_in[F * l + 1024 * phi:F * l + 1024 * (phi + 1)])
                    for rb in range(NRB):
                        for o2 in range(2):
                            ps = ps_big.tile([128, 512], F32, tag="big")
                            for c in range(NC_):
                                nc.tensor.matmul(
                                    ps, yT[:, 8 * phi + c, 128 * rb:128 * (rb + 1)],
                                    w2_sb[:, c, 512 * o2:512 * (o2 + 1)],
                                    start=(c == 0), stop=(c == NC_ - 1))
                            hsl = h_sb[:, rb, 512 * o2:512 * (o2 + 1)]
                            nc.vector.tensor_add(hsl, hsl, ps)
                b2_bc = prm.tile([128, D], F32, tag="b2")
                nc.sync.dma_start(b2_bc, bcast_ap(b2_in[ds(D * l, D)]))
                for rb in range(NRB):
                    nc.vector.tensor_add(h_sb[:, rb, :], h_sb[:, rb, :], b2_bc)

            # ---- final LN + decoder ----
            xTf = xtp.tile([128, NC_, RLOC], BF, tag="xt")
            layernorm_to_xT(lnfg_in[:], lnfb_in[:], xTf)
            out_sb = res.tile([128, NRB, OUT], F32)
            for rb in range(NRB):
                ps = ps_big.tile([128, OUT], F32, tag="big")
                for c in range(NC_):
                    nc.tensor.matmul(ps, xTf[:, c, 128 * rb:128 * (rb + 1)],
                                     wd_sb[:, c, :], start=(c == 0), stop=(c == NC_ - 1))
                nc.vector.tensor_add(out_sb[:, rb, :], bd_bc, ps)
            nc.sync.dma_start(out_p.rearrange("(rb p) n -> p rb n", p=128), out_sb)

    nc.compile()
    return nc


_PROGRAM = None


def _get_program():
    global _PROGRAM
    if _PROGRAM is None:
        _PROGRAM = _build_program()
    return _PROGRAM


def _bf(x):
    return np.ascontiguousarray(np.asarray(x, np.float32)).astype(ml_dtypes.bfloat16)


def _prep_inputs(inputs):
    """Host-side sharding: build the per-core input maps."""
    I = {k: np.asarray(v) for k, v in inputs.items()}

    wq = _bf(I["wq"].reshape(L * D, D))
    wk = _bf(I["wk"].reshape(L * D, D))
    wv = _bf(I["wv"].reshape(L * D, D))
    wo = _bf(I["wo"].reshape(L * D, D))
    w1 = _bf(I["w1"].reshape(L * D, F))
    w2 = _bf(I["w2"].reshape(L * F, D))
    b1 = np.asarray(I["b1"].reshape(L * F), np.float32)
    b2 = np.asarray(I["b2"].reshape(L * D), np.float32)
    ln1g = np.asarray(I["ln1_g"].reshape(L * D), np.float32)
    ln1b = np.asarray(I["ln1_b"].reshape(L * D), np.float32)
    ln2g = np.asarray(I["ln2_g"].reshape(L * D), np.float32)
    ln2b = np.asarray(I["ln2_b"].reshape(L * D), np.float32)
    lnfg = np.asarray(I["lnf_g"], np.float32)
    lnfb = np.asarray(I["lnf_b"], np.float32)
    wd = _bf(I["wd"])
    bd = np.asarray(I["bd"], np.float32)

    # augmented embedding table [64, D]
    wa = np.zeros((64, D), np.float32)
    wa[0:V1] = I["emb_cat1"]
    wa[V1:V1 + V2] = I["emb_cat2"]
    wa[48] = I["w_num1"][0]
    wa[49] = I["w_num2"][0]
    wa[50] = I["bos"][0, 0]
    wa = _bf(wa)

    pos_emb = np.asarray(I["pos_emb"], np.float32)
    cat1 = np.asarray(I["tgt_cat1"])
    cat2 = np.asarray(I["tgt_cat2"])
    num1 = np.asarray(I["tgt_num1"], np.float32)
    num2 = np.asarray(I["tgt_num2"], np.float32)

    in_maps = []
    shared = dict(wq=wq, wk=wk, wv=wv, wo=wo, w1=w1, w2=w2, b1=b1, b2=b2,
                  ln1g=ln1g, ln1b=ln1b, ln2g=ln2g, ln2b=ln2b,
                  lnfg=lnfg, lnfb=lnfb, wd=wd, bd=bd, wa=wa)
    for c in range(NCORES):
        b, parity = c // 2, c % 2
        grows = np.concatenate([np.arange(128 * g, 128 * (g + 1))
                                for g in BLOCKS[parity]])        # [512] global rows
        grows_rem = np.concatenate([np.arange(128 * g, 128 * (g + 1))
                                    for g in BLOCKS[1 - parity]])
        # embedding selector EaT [64, 512]
        eat = np.zeros((64, RLOC), np.float32)
        for r, g in enumerate(grows):
            if g == 0:
                eat[50, r] = 1.0
            else:
                t = g - 1
                eat[cat1[b, t], r] = 1.0
                eat[V1 + cat2[b, t], r] = 1.0
                eat[48, r] = num1[b, t, 0]
                eat[49, r] = num2[b, t, 0]
        # shifted positional embedding [512, D]
        pos = np.zeros((RLOC, D), np.float32)
        nz = grows > 0
        pos[nz] = pos_emb[grows[nz] - 1]
        # binary causal masks [128, 8, 512]: slots 0..3 local rows, 4..7 partner rows
        mask = np.zeros((128, 8, RLOC), np.float32)
        rk_loc = grows.reshape(4, 128).transpose(1, 0)          # [p, jl]
        rk_rem = grows_rem.reshape(4, 128).transpose(1, 0)
        mask[:, 0:4, :] = (rk_loc[:, :, None] <= grows[None, None, :])
        mask[:, 4:8, :] = (rk_rem[:, :, None] <= grows[None, None, :])
        in_maps.append(dict(shared,
                            eat=_bf(eat), pos=pos, masks=_bf(mask)))
    return in_maps


def _unshard_output(results):
    out = np.zeros((B, S, OUT), np.float32)
    for c in range(NCORES):
        b, parity = c // 2, c % 2
        grows = np.concatenate([np.arange(128 * g, 128 * (g + 1))
                                for g in BLOCKS[parity]])
        out[b, grows] = results[c]["out"]
    return out


def kernel(**inputs):
    nc = _get_program()
    in_maps = _prep_inputs(inputs)
    res = run_bass_kernel_spmd(nc, in_maps, core_ids=list(range(NCORES)))
    return _unshard_output(res.results)


def run_traced(inputs):
    """Like kernel() but with NTFF tracing; returns (output, BassKernelResults)."""
    nc = _get_program()
    in_maps = _prep_inputs(inputs)
    res = run_bass_kernel_spmd(nc, in_maps, core_ids=list(range(NCORES)),
                               trace=True, trace_cores=list(range(NCORES)))
    return _unshard_output(res.results), res



# revision 62
# speedup vs baseline: 1.3122x; 1.3122x over previous
"""Trainium2 Bass kernel for nn_AutoReg (4-layer dense transformer, teacher forcing).

Sharding across 8 NeuronCores: data-parallel over batch (B=4 -> 4 core pairs),
sequence-split within each pair using INTERLEAVED row blocks (even core owns
global 128-row blocks [0,2,4,6], odd core [1,3,5,7]).  That makes causal
visibility index-triangular and identical on every core: for both the local
and the remote key set, key-block i is visible to query-block j iff i <= j,
with the i == j sub-block handled by a data mask (within-block triangle for
the local pass; all-ones/all-zeros by parity for the remote pass).  Fully
masked blocks are never computed, cutting score/AV/exp work ~40%.

Instead of exchanging K/V (2 MB), each pair AllGathers the LN1 output xT
(1 MB bf16) right after LN1; the partner's K/V are recomputed locally from
the gathered xT while the collective is in flight behind the local
K/V/Q projections and the local attention pass.

Math: bf16 matmul inputs with fp32 PSUM accumulation; LayerNorm, softmax and
the residual stream in fp32.  Scores are computed transposed (sT[rk, rq]) so
the softmax denominator comes out of the AV matmul via an appended ones
column; normalization uses a fast approximate reciprocal plus a K=1
broadcast matmul.
"""

import numpy as np
import ml_dtypes

import concourse.bass as bass
import concourse.bacc as bacc
import concourse.mybir as mybir
import concourse.tile as tile
from concourse.bass import ds
from concourse.bass_utils import run_bass_kernel_spmd
from concourse.masks import make_identity

# Model dims (hardcoded per the problem spec)
L, B, S, D, H, F = 4, 4, 1024, 1024, 16, 4096
V1, V2, OUT = 32, 16, 50
HD = D // H            # 64
NCORES = 8
RLOC = 512             # local rows per core
NRB = RLOC // 128      # 4 local row blocks
NC_ = D // 128         # 8 D-chunks
NFO = F // 128         # 32 F-chunks
SCALE = 1.0 / np.sqrt(HD)

# interleaved global row-block assignment: even cores own blocks [0,2,4,6],
# odd cores [1,3,5,7] -> causal block visibility is index-triangular on
# every core for both local and remote key sets.
BLOCKS = {0: [0, 2, 4, 6], 1: [1, 3, 5, 7]}

BF = mybir.dt.bfloat16
F32 = mybir.dt.float32
FP8 = mybir.dt.float8e4

SEG_X = D * RLOC             # elems: one core's xT contribution
X_FP8 = False                # exchange dtype: fp8 (scaled) or plain bf16
X8S = 8.0                    # fp8 exchange scale (xT8 = 8 * xT)
XDT_ = mybir.dt.float8e4


def _build_program():
    nc = bacc.Bacc("TRN2", target_bir_lowering=False)

    # ---- DRAM parameters (per-core inputs) ----
    eat_in = nc.declare_dram_parameter("eat", [64, RLOC], BF, isOutput=False)
    wa_in = nc.declare_dram_parameter("wa", [64, D], BF, isOutput=False)
    pos_in = nc.declare_dram_parameter("pos", [RLOC, D], F32, isOutput=False)
    masks_in = nc.declare_dram_parameter("masks", [128, 2, 128], BF, isOutput=False)
    wq_in = nc.declare_dram_parameter("wq", [L * D, D], BF, isOutput=False)
    wk_in = nc.declare_dram_parameter("wk", [L * D, D], BF, isOutput=False)
    wv_in = nc.declare_dram_parameter("wv", [L * D, D], BF, isOutput=False)
    wo_in = nc.declare_dram_parameter("wo", [L * D, D], BF, isOutput=False)
    w1_in = nc.declare_dram_parameter("w1", [L * D, F], BF, isOutput=False)
    w2_in = nc.declare_dram_parameter("w2", [L * F, D], BF, isOutput=False)
    b1_in = nc.declare_dram_parameter("b1", [L * F], F32, isOutput=False)
    b2_in = nc.declare_dram_parameter("b2", [L * D], BF, isOutput=False)
    # LN gains are folded into the consuming weights host-side; these hold
    # the folded biases b' = b/g added to the normalized x.
    ln1b_in = nc.declare_dram_parameter("ln1b", [L * D], F32, isOutput=False)
    ln2b_in = nc.declare_dram_parameter("ln2b", [L * D], F32, isOutput=False)
    lnfb_in = nc.declare_dram_parameter("lnfb", [D], F32, isOutput=False)
    wd_in = nc.declare_dram_parameter("wd", [D, OUT], BF, isOutput=False)
    bd_in = nc.declare_dram_parameter("bd", [OUT], F32, isOutput=False)
    out_p = nc.declare_dram_parameter("out", [RLOC, OUT], F32, isOutput=True)

    def bcast_ap(src_ap, p=128):
        """Partition-broadcast view of a 1-D DRAM AP."""
        return bass.AP(tensor=src_ap.tensor, offset=src_ap.offset,
                       ap=[[0, p]] + [list(x) for x in src_ap.ap])

    AF = mybir.ActivationFunctionType
    ALU = mybir.AluOpType

    with tile.TileContext(nc) as tc:
        with tc.tile_pool(name="res", bufs=1) as res, \
             tc.tile_pool(name="wbig", bufs=2) as wbig, \
             tc.tile_pool(name="yt", bufs=1) as ytp, \
             tc.tile_pool(name="xt", bufs=2) as xtp, \
             tc.tile_pool(name="expp", bufs=3) as expp, \
             tc.tile_pool(name="xc", bufs=1) as xcp, \
             tc.tile_pool(name="prm", bufs=2) as prm, \
             tc.tile_pool(name="sm", bufs=4) as sm, \
             tc.tile_pool(name="dr", bufs=1, space="DRAM") as dr, \
             tc.tile_pool(name="ps_big", bufs=2, space="PSUM") as ps_big, \
             tc.tile_pool(name="ps_s", bufs=4, space="PSUM") as ps_s, \
             tc.tile_pool(name="ps_av", bufs=2, space="PSUM") as ps_av:

            # ---- resident tiles ----
            h_sb = res.tile([128, NRB, D], F32)            # residual stream
            kst = res.tile([128, NC_, RLOC], BF)           # own K^T
            kT_rem = res.tile([128, NC_, RLOC], BF)        # partner K^T
            # V tiles: each head stores [values(64) | ones(64)] contiguously;
            # the ones half lands the softmax denominator replicated on psum
            # partitions 64..127 of the AV output.
            v_loc = res.tile([128, NRB, H, 2 * HD], BF)    # own V | ones
            v_rem = res.tile([128, NRB, H, 2 * HD], BF)    # partner V | ones
            qT_sb = res.tile([128, NC_, RLOC], BF)
            oT_sb = res.tile([128, NC_, RLOC], BF)
            p1_sb = res.tile([128, H, RLOC], BF)           # pass-1 partial (o|sum)
            # embedding-only tiles: borrow pool slots that are first used
            # later (yT in FFN1, xT rotation), so they cost no extra SBUF
            eat_sb = ytp.tile([64, RLOC], BF, tag="yt")
            wa_sb = xtp.tile([64, D], BF, tag="xt")
            ident = res.tile([128, 128], F32)
            ident_bf = res.tile([128, 128], BF)
            wd_sb = res.tile([128, NC_, OUT], BF)
            bd_bc = res.tile([128, OUT], F32)
            eps_sb = res.tile([128, 1], F32)
            mask_sb = res.tile([128, 2, 128], BF)          # [tri | remote-diag]

            make_identity(nc, ident)
            nc.vector.tensor_copy(ident_bf, ident)
            nc.vector.memset(eps_sb, 1e-6)
            nc.vector.memset(v_loc[:, :, :, HD:2 * HD], 1.0)
            nc.vector.memset(v_rem[:, :, :, HD:2 * HD], 1.0)
            nc.sync.dma_start(eat_sb, eat_in[:, :])
            nc.sync.dma_start(wa_sb, wa_in[:, :])
            nc.sync.dma_start(mask_sb, masks_in[:, :, :])
            nc.sync.dma_start(wd_sb, wd_in.rearrange("(c p) n -> p c n", p=128))
            nc.sync.dma_start(bd_bc, bcast_ap(bd_in[:]))

            # dynamic base: partner's segment offset in the pair AllGather output
            pid = nc.sync.partition_id()
            par = pid - (pid // 2) * 2
            rem_base = (1 - par) * SEG_X

            # ---- embedding: h = EaT^T @ Wa + pos ----
            pos_sb = wbig.tile([128, NRB, D], F32, tag="w2mb")
            nc.sync.dma_start(pos_sb, pos_in.rearrange("(rb p) d -> p rb d", p=128))
            for rb in range(NRB):
                for o2 in range(2):
                    ps = ps_big.tile([128, 512], F32, tag="big")
                    nc.tensor.matmul(ps, eat_sb[:, 128 * rb:128 * (rb + 1)],
                                     wa_sb[:, 512 * o2:512 * (o2 + 1)],
                                     start=True, stop=True)
                    nc.vector.tensor_add(h_sb[:, rb, 512 * o2:512 * (o2 + 1)],
                                         pos_sb[:, rb, 512 * o2:512 * (o2 + 1)], ps)

            # warm up the ACT exp/ln table set
            warm = sm.tile([128, 1], F32, tag="s1")
            nc.vector.memset(warm, 1.0)
            nc.scalar.activation(warm, warm, AF.Ln, bias=eps_sb, scale=1.0)
            nc.scalar.activation(warm, warm, AF.Exp, bias=0.0, scale=-0.5)

            def ln_params(b_src, with8=False):
                b_sb = prm.tile([128, NC_], F32, tag="lnb")
                nc.sync.dma_start(b_sb, b_src.rearrange("(c p) -> p c", p=128))
                return b_sb, with8

            def ln_rb(b_sb, xT, rb, xT8=None):
                """LN of one 128-row block of h into xT[:, :, 128*rb:...].

                LN gains are pre-folded into the consuming weights; b_sb holds
                b/g, applied here by the ACT copy-out.  When xT8 is given, a
                second fp8 copy scaled by X8S is written for the exchange."""
                stats = sm.tile([128, 2, 6], F32, tag="st")
                nc.vector.bn_stats(stats[:, 0, :], h_sb[:, rb, 0:512])
                nc.vector.bn_stats(stats[:, 1, :], h_sb[:, rb, 512:1024])
                mv = sm.tile([128, 2], F32, tag="mv")
                nc.vector.bn_aggr(mv, stats)
                rstd = sm.tile([128, 1], F32, tag="rstd")
                nc.scalar.activation(rstd, mv[:, 1:2], AF.Ln, bias=eps_sb, scale=1.0)
                nc.scalar.activation(rstd, rstd, AF.Exp, bias=0.0, scale=-0.5)
                xc = xcp.tile([128, D], BF, tag="xc")
                nc.vector.tensor_scalar(xc, h_sb[:, rb, :], mv[:, 0:1], rstd,
                                        ALU.subtract, ALU.mult)
                for c in range(NC_):
                    tp = ps_s.tile([128, 128], BF, tag="s")
                    nc.tensor.transpose(tp, xc[:, 128 * c:128 * (c + 1)], ident_bf)
                    nc.vector.tensor_scalar_add(xT[:, c, 128 * rb:128 * (rb + 1)],
                                                tp, b_sb[:, c:c + 1])
                if xT8 is not None and X_FP8:
                    nc.vector.tensor_scalar_mul(
                        xT8[:, :, 128 * rb:128 * (rb + 1)],
                        xT[:, :, 128 * rb:128 * (rb + 1)], X8S)

            def layernorm_to_xT(b_src, xT):
                b_sb, _ = ln_params(b_src)
                for rb in range(NRB):
                    ln_rb(b_sb, xT, rb)

            def load_w(src2d, tag="w2mb", split=1):
                # split>1 issues the load as multiple DMAs over the c-chunk
                # dim, so consumers of early chunks unblock sooner
                w = wbig.tile([128, NC_, src2d.shape[1]], BF, tag=tag)
                if split == 1:
                    nc.sync.dma_start(w, src2d.rearrange("(c p) n -> p c n", p=128))
                else:
                    cs = NC_ // split
                    for s in range(split):
                        nc.sync.dma_start(
                            w[:, s * cs:(s + 1) * cs, :],
                            src2d[128 * s * cs:128 * (s + 1) * cs, :].rearrange(
                                "(c p) n -> p c n", p=128))
                return w

            def proj_kT(xT_src, wsb, kdst, scale=None, rb_split=False,
                        halves=False):
                """K^T-style projection: kdst[:, o, :] = (w^T x)[128o:.., all rows].

                rb_split runs the moving operand in 128-col blocks so the
                matmuls start as soon as the producing LN finishes each row
                block (used right after the fused FFN2/LN tail).  halves
                additionally evacuates per 256-col half so consumers unblock
                as each half-collective read lands."""
                for o in range(NC_):
                    ps = ps_big.tile([128, 512], F32, tag="big")
                    if rb_split or halves:
                        for rb in range(NRB):
                            sl = slice(128 * rb, 128 * (rb + 1))
                            for c in range(NC_):
                                nc.tensor.matmul(ps[:, sl],
                                                 wsb[:, c, 128 * o:128 * (o + 1)],
                                                 xT_src[:, c, sl], start=(c == 0),
                                                 stop=(c == NC_ - 1))
                            if halves and rb % 2 == 1:
                                hs = slice(256 * (rb // 2), 256 * (rb // 2 + 1))
                                nc.vector.tensor_copy(kdst[:, o, hs], ps[:, hs])
                        if halves:
                            continue
                    else:
                        for c in range(NC_):
                            nc.tensor.matmul(ps, wsb[:, c, 128 * o:128 * (o + 1)],
                                             xT_src[:, c, :], start=(c == 0),
                                             stop=(c == NC_ - 1))
                    if scale is None:
                        nc.vector.tensor_copy(kdst[:, o, :], ps)
                    else:
                        nc.vector.tensor_scalar_mul(kdst[:, o, :], ps, scale)

            def proj_v(xT_src, wsb, vdst, scale=None):
                """Token-major V projection into vdst[..., 0:HD]."""
                for rb in range(NRB):
                    for o2 in range(2):
                        ps = ps_big.tile([128, 512], F32, tag="big")
                        for c in range(NC_):
                            nc.tensor.matmul(ps, xT_src[:, c, 128 * rb:128 * (rb + 1)],
                                             wsb[:, c, 512 * o2:512 * (o2 + 1)],
                                             start=(c == 0), stop=(c == NC_ - 1))
                        dst = vdst[:, rb, 8 * o2:8 * (o2 + 1), 0:HD]
                        src = ps.rearrange("p (hh e) -> p hh e", hh=8)
                        if scale is None:
                            nc.vector.tensor_copy(dst, src)
                        else:
                            nc.vector.tensor_scalar_mul(dst, src, scale)

            def attn_pass_pair(i, kt, vt, mslot, av0, av1):
                """One attention pass for head pair (2i, 2i+1), causal-skipped.

                Key block kb contributes only to query cols [128*kb:512]; the
                kb==qb sub-block is masked by mask_sb[:, mslot, :].  The two
                heads' score matmuls contract over disjoint partition halves
                of kT/qT, so the PE runs them concurrently."""
                h0, h1 = 2 * i, 2 * i + 1
                expT0 = expp.tile([128, 4, RLOC], BF, tag="exp")
                expT1 = expp.tile([128, 4, RLOC], BF, tag="exp")
                for kb in range(4):
                    q0 = 128 * kb
                    st0 = ps_s.tile([128, RLOC], F32, tag="s")
                    st1 = ps_s.tile([128, RLOC], F32, tag="s")
                    nc.tensor.matmul(st0[:, q0:], kt[0:64, i, q0:q0 + 128],
                                     qT_sb[0:64, i, q0:], start=True, stop=True)
                    nc.tensor.matmul(st1[:, q0:], kt[64:128, i, q0:q0 + 128],
                                     qT_sb[64:128, i, q0:], start=True, stop=True)
                    nc.scalar.activation(expT0[:, kb, q0:], st0[:, q0:], AF.Exp)
                    nc.gpsimd.tensor_tensor(expT0[:, kb, q0:q0 + 128],
                                            expT0[:, kb, q0:q0 + 128],
                                            mask_sb[:, mslot, :], ALU.mult)
                    nc.scalar.activation(expT1[:, kb, q0:], st1[:, q0:], AF.Exp)
                    nc.gpsimd.tensor_tensor(expT1[:, kb, q0:q0 + 128],
                                            expT1[:, kb, q0:q0 + 128],
                                            mask_sb[:, mslot, :], ALU.mult)
                for kb in range(4):
                    q0 = 128 * kb
                    nc.tensor.matmul(av0[:, q0:], vt[:, kb, h0, :],
                                     expT0[:, kb, q0:], start=(kb == 0),
                                     stop=(kb == 3))
                    nc.tensor.matmul(av1[:, q0:], vt[:, kb, h1, :],
                                     expT1[:, kb, q0:], start=(kb == 0),
                                     stop=(kb == 3))

            # LN1 of layer 0 (later layers' LN1 is fused into the FFN2 tail)
            xT = xtp.tile([128, NC_, RLOC], BF, tag="xt")
            xT8 = xtp.tile([128, NC_, RLOC], FP8, tag="x8", name="xT8_p") if X_FP8 else xT
            b_sb0, _ = ln_params(ln1b_in[ds(0, D)])
            with tc.high_priority():
                for rb in range(NRB):
                    ln_rb(b_sb0, xT, rb, xT8)

            for l in range(L):
                # per-layer xT exchange bounce buffers (fp8, scaled by X8S),
                # split into two row-halves so the first half-collective lands
                # ~30us earlier and the remote projections stream in per half
                HSEG = SEG_X // 2
                XDT = FP8 if X_FP8 else BF
                xsrc = xT8 if X_FP8 else xT
                x_in = [dr.tile([HSEG], XDT, tag=f"xin{h}", name=f"x_in_{l}_{h}")
                        for h in range(2)]
                x_all = [dr.tile([2 * HSEG], XDT, tag=f"xall{h}",
                                 name=f"x_all_{l}_{h}") for h in range(2)]

                # ---- AllGather fp8 xT across the pair (Pool queue: the x-out
                # writes and the collectives stay ordered there without
                # blocking the SP weight-load queue) ----
                for hh in range(2):
                    nc.gpsimd.dma_start(
                        x_in[hh].rearrange("(c p r) -> p c r", c=NC_, p=128),
                        xsrc[:, :, 256 * hh:256 * (hh + 1)])
                    nc.gpsimd.collective_compute(
                        "AllGather", ALU.bypass,
                        replica_groups=[[0, 1], [2, 3], [4, 5], [6, 7]],
                        ins=[x_in[hh][:]], outs=[x_all[hh][:]])

                # ---- local K/Q/V projections (overlap the AllGather) ----
                wk_sb = load_w(wk_in[D * l:D * (l + 1)], split=2)
                proj_kT(xT, wk_sb, kst, rb_split=True)
                wq_sb = load_w(wq_in[D * l:D * (l + 1)], split=2)
                proj_kT(xT, wq_sb, qT_sb, rb_split=True)
                wv_sb = load_w(wv_in[D * l:D * (l + 1)], split=2)
                proj_v(xT, wv_sb, v_loc)

                # ---- attention pass 1: own K/V (no AllGather dependency).
                # high_priority makes the scheduler prefer pass instructions
                # whenever ready, so the V projection (and, once the ~45us
                # fp8 collective lands mid-pass, the remote projections) fill
                # the PE bubbles left while ACT runs the exps.
                with tc.high_priority():
                    for i in range(H // 2):
                        pa = ps_av.tile([128, RLOC], F32, tag="av")
                        pb = ps_av.tile([128, RLOC], F32, tag="av")
                        attn_pass_pair(i, kst, v_loc, 0, pa, pb)
                        nc.vector.tensor_copy(p1_sb[:, 2 * i, :], pa)
                        nc.vector.tensor_copy(p1_sb[:, 2 * i + 1, :], pb)

                # ---- fetch partner fp8 xT, recompute its K/V (weight loads
                # are issued before the x_all read so the SP DMA queue isn't
                # head-of-line blocked on the collective) ----
                wk_sb2 = load_w(wk_in[D * l:D * (l + 1)], split=2)
                wv_sb2 = load_w(wv_in[D * l:D * (l + 1)], split=2)
                xTr_bf = xtp.tile([128, NC_, RLOC], BF, tag="xt")
                if X_FP8:
                    xT8_rem = xtp.tile([128, NC_, RLOC], FP8, tag="x8")
                for hh in range(2):
                    hs = slice(256 * hh, 256 * (hh + 1))
                    if X_FP8:
                        nc.sync.dma_start(
                            xT8_rem[:, :, hs],
                            x_all[hh][ds(rem_base // 2, HSEG)].rearrange(
                                "(c p r) -> p c r", c=NC_, p=128))
                        # upconvert+descale to bf16 on DVE (per chunk,
                        # prioritized, so the remote projections pipeline):
                        # PE mixed fp8*bf16 matmuls are avoided
                        with tc.high_priority():
                            for c in range(NC_):
                                nc.vector.tensor_scalar_mul(xTr_bf[:, c, hs],
                                                            xT8_rem[:, c, hs],
                                                            1.0 / X8S)
                    else:
                        nc.sync.dma_start(
                            xTr_bf[:, :, hs],
                            x_all[hh][ds(rem_base // 2, HSEG)].rearrange(
                                "(c p r) -> p c r", c=NC_, p=128))
                proj_kT(xTr_bf, wk_sb2, kT_rem, halves=True)
                proj_v(xTr_bf, wv_sb2, v_rem)

                # ---- attention pass 2: partner K/V, combine, normalize.
                # The AV matmul lands the softmax denominator replicated on
                # partitions 64..127 (ones slot in v), so normalization is
                # add + reciprocal + multiply on DVE, no broadcast matmul.
                def finalize_head(h, av2):
                    hp, ho = 64 * (h % 2), h // 2
                    nc.vector.tensor_tensor(av2, av2, p1_sb[:, h, :], ALU.add)
                    # stage the sums to SBUF before the reciprocal: its
                    # bitwise fast-inverse seed must see fp32 bit layout,
                    # which a raw PSUM read is not guaranteed to provide
                    rcp = sm.tile([64, RLOC], F32, tag="rcp", bufs=2)
                    nc.vector.tensor_copy(rcp, av2[64:128, :])
                    nc.vector.reciprocal_approx_fast(rcp, rcp)
                    nc.vector.tensor_tensor(oT_sb[hp:hp + 64, ho, :],
                                            av2[0:64, :], rcp, ALU.mult)

                wo_sb = load_w(wo_in[D * l:D * (l + 1)], split=2)
                with tc.high_priority():
                    for i in range(H // 2):
                        pa = ps_av.tile([128, RLOC], F32, tag="av")
                        pb = ps_av.tile([128, RLOC], F32, tag="av")
                        attn_pass_pair(i, kT_rem, v_rem, 1, pa, pb)
                        finalize_head(2 * i, pa)
                        finalize_head(2 * i + 1, pb)

                # ---- output projection + residual ----
                for rb in range(NRB):
                    for o2 in range(2):
                        ps = ps_big.tile([128, 512], F32, tag="big")
                        for c in range(NC_):
                            nc.tensor.matmul(ps, oT_sb[:, c, 128 * rb:128 * (rb + 1)],
                                             wo_sb[:, c, 512 * o2:512 * (o2 + 1)],
                                             start=(c == 0), stop=(c == NC_ - 1))
                        hsl = h_sb[:, rb, 512 * o2:512 * (o2 + 1)]
                        nc.vector.tensor_add(hsl, hsl, ps)

                # ---- LN2 -> xT2 ----
                xT2 = xtp.tile([128, NC_, RLOC], BF, tag="xt")
                layernorm_to_xT(ln2b_in[ds(D * l, D)], xT2)

                # ---- FFN1: yT = relu(w1^T x + b1) ----
                b1_sb = prm.tile([128, NFO], F32, tag="b1")
                nc.sync.dma_start(b1_sb, b1_in[ds(F * l, F)].rearrange("(o p) -> p o", p=128))
                yT = ytp.tile([128, NFO, RLOC], BF, tag="yt")
                for phi in range(4):
                    w1_sb = load_w(w1_in[D * l:D * (l + 1), 1024 * phi:1024 * (phi + 1)], split=2)
                    for fo in range(8):
                        fg = 8 * phi + fo
                        ps = ps_big.tile([128, 512], F32, tag="big")
                        for rb in range(NRB):
                            sl = slice(128 * rb, 128 * (rb + 1))
                            for c in range(NC_):
                                nc.tensor.matmul(ps[:, sl],
                                                 w1_sb[:, c, 128 * fo:128 * (fo + 1)],
                                                 xT2[:, c, sl], start=(c == 0),
                                                 stop=(c == NC_ - 1))
                        nc.scalar.activation(yT[:, fg, :], ps, AF.Relu,
                                             bias=b1_sb[:, fg:fg + 1], scale=1.0)

                # ---- FFN2: h += yT^T @ w2 (+ b2); the phi==3 tail finalizes
                # each row block and immediately runs the next LN1 (or the
                # final LN) so the next collective can dispatch at FFN2-end.
                b2_bc = prm.tile([128, D], BF, tag="b2", bufs=1)
                nc.sync.dma_start(b2_bc, bcast_ap(b2_in[ds(D * l, D)]))
                last = l == L - 1
                bn_sb, _ = ln_params(
                    lnfb_in[:] if last else ln1b_in[ds(D * (l + 1), D)])
                xT_next = xtp.tile([128, NC_, RLOC], BF, tag="xt")
                xT8_next = None if last else (
                    xtp.tile([128, NC_, RLOC], FP8, tag="x8",
                             name=f"xT8_n{l}") if X_FP8
                    else xT_next)
                # b2 is data-independent of the FFN2 matmuls: add it up front
                # so the phi==3 tail chain (h -> LN -> xT8 -> collective) is
                # as short as possible.
                for rb in range(NRB):
                    nc.vector.tensor_add(h_sb[:, rb, :], h_sb[:, rb, :], b2_bc)
                # half-major: rows 0:256 finish all phis first, so their LN
                # tail (gating the first half-collective of the next layer)
                # starts ~20us earlier; w2 is loaded twice per layer for this
                for half in range(2):
                    for phi in range(4):
                        w2_sb = load_w(
                            w2_in[F * l + 1024 * phi:F * l + 1024 * (phi + 1)],
                            split=2)
                        for rb in (2 * half, 2 * half + 1):
                            for o2 in range(2):
                                ps = ps_big.tile([128, 512], F32, tag="big")
                                for c in range(NC_):
                                    nc.tensor.matmul(
                                        ps,
                                        yT[:, 8 * phi + c, 128 * rb:128 * (rb + 1)],
                                        w2_sb[:, c, 512 * o2:512 * (o2 + 1)],
                                        start=(c == 0), stop=(c == NC_ - 1))
                                hsl = h_sb[:, rb, 512 * o2:512 * (o2 + 1)]
                                nc.vector.tensor_add(hsl, hsl, ps)
                            if phi == 3:
                                with tc.high_priority():
                                    ln_rb(bn_sb, xT_next, rb, xT8_next)
                xT = xT_next
                xT8 = xT8_next

            # ---- decoder (xT now holds the final-LN output) ----
            xTf = xT
            out_sb = res.tile([128, NRB, OUT], F32)
            for rb in range(NRB):
                ps = ps_big.tile([128, OUT], F32, tag="big")
                for c in range(NC_):
                    nc.tensor.matmul(ps, xTf[:, c, 128 * rb:128 * (rb + 1)],
                                     wd_sb[:, c, :], start=(c == 0), stop=(c == NC_ - 1))
                nc.vector.tensor_add(out_sb[:, rb, :], bd_bc, ps)
            nc.sync.dma_start(out_p.rearrange("(rb p) n -> p rb n", p=128), out_sb)

    nc.compile()
    return nc


_PROGRAM = None


def _get_program():
    global _PROGRAM
    if _PROGRAM is None:
        _PROGRAM = _build_program()
    return _PROGRAM


def _bf(x):
    return np.ascontiguousarray(np.asarray(x, np.float32)).astype(ml_dtypes.bfloat16)


def _prep_inputs(inputs):
    """Host-side sharding: build the per-core input maps."""
    I = {k: np.asarray(v) for k, v in inputs.items()}

    # Fold LN gains into the consuming weights (exact for any g):
    #   LN(x) @ W = (xhat + b/g) @ (diag(g) W); the kernel applies b/g.
    ln1g = np.asarray(I["ln1_g"], np.float32)          # [L, D]
    ln2g = np.asarray(I["ln2_g"], np.float32)
    lnfg = np.asarray(I["lnf_g"], np.float32)

    def safediv(b, g):
        return np.where(g != 0.0, b / np.where(g != 0.0, g, 1.0), 0.0)

    wq = _bf((np.asarray(I["wq"], np.float32) * ln1g[:, :, None]
              * np.float32(SCALE)).reshape(L * D, D))
    wk = _bf((np.asarray(I["wk"], np.float32) * ln1g[:, :, None]).reshape(L * D, D))
    wv = _bf((np.asarray(I["wv"], np.float32) * ln1g[:, :, None]).reshape(L * D, D))
    wo = _bf(I["wo"].reshape(L * D, D))
    w1 = _bf((np.asarray(I["w1"], np.float32) * ln2g[:, :, None]).reshape(L * D, F))
    w2 = _bf(I["w2"].reshape(L * F, D))
    b1 = np.asarray(I["b1"].reshape(L * F), np.float32)
    b2 = _bf(I["b2"].reshape(L * D))
    ln1b = safediv(np.asarray(I["ln1_b"], np.float32), ln1g).reshape(L * D)
    ln2b = safediv(np.asarray(I["ln2_b"], np.float32), ln2g).reshape(L * D)
    lnfb = safediv(np.asarray(I["lnf_b"], np.float32), lnfg)
    wd = _bf(np.asarray(I["wd"], np.float32) * lnfg[:, None])
    bd = np.asarray(I["bd"], np.float32)

    # augmented embedding table [64, D]
    wa = np.zeros((64, D), np.float32)
    wa[0:V1] = I["emb_cat1"]
    wa[V1:V1 + V2] = I["emb_cat2"]
    wa[48] = I["w_num1"][0]
    wa[49] = I["w_num2"][0]
    wa[50] = I["bos"][0, 0]
    wa = _bf(wa)

    pos_emb = np.asarray(I["pos_emb"], np.float32)
    cat1 = np.asarray(I["tgt_cat1"])
    cat2 = np.asarray(I["tgt_cat2"])
    num1 = np.asarray(I["tgt_num1"], np.float32)
    num2 = np.asarray(I["tgt_num2"], np.float32)

    # masks [128, 2, 128]: slot 0 within-block triangle; slot 1 remote-diag
    # (all-zeros on even cores, all-ones on odd cores)
    tri = (np.arange(128)[:, None] <= np.arange(128)[None, :]).astype(np.float32)

    in_maps = []
    shared = dict(wq=wq, wk=wk, wv=wv, wo=wo, w1=w1, w2=w2, b1=b1, b2=b2,
                  ln1b=ln1b, ln2b=ln2b, lnfb=lnfb, wd=wd, bd=bd, wa=wa)
    for c in range(NCORES):
        b, parity = c // 2, c % 2
        grows = np.concatenate([np.arange(128 * g, 128 * (g + 1))
                                for g in BLOCKS[parity]])        # [512] global rows
        # embedding selector EaT [64, 512]
        eat = np.zeros((64, RLOC), np.float32)
        for r, g in enumerate(grows):
            if g == 0:
                eat[50, r] = 1.0
            else:
                t = g - 1
                eat[cat1[b, t], r] = 1.0
                eat[V1 + cat2[b, t], r] = 1.0
                eat[48, r] = num1[b, t, 0]
                eat[49, r] = num2[b, t, 0]
        # shifted positional embedding [512, D]
        pos = np.zeros((RLOC, D), np.float32)
        nz = grows > 0
        pos[nz] = pos_emb[grows[nz] - 1]
        mask = np.zeros((128, 2, 128), np.float32)
        mask[:, 0, :] = tri
        mask[:, 1, :] = float(parity)
        in_maps.append(dict(shared,
                            eat=_bf(eat), pos=pos, masks=_bf(mask)))
    return in_maps


def _unshard_output(results):
    out = np.zeros((B, S, OUT), np.float32)
    for c in range(NCORES):
        b, parity = c // 2, c % 2
        grows = np.concatenate([np.arange(128 * g, 128 * (g + 1))
                                for g in BLOCKS[parity]])
        out[b, grows] = results[c]["out"]
    return out


def kernel(**inputs):
    nc = _get_program()
    in_maps = _prep_inputs(inputs)
    res = run_bass_kernel_spmd(nc, in_maps, core_ids=list(range(NCORES)))
    return _unshard_output(res.results)


def run_traced(inputs):
    """Like kernel() but with NTFF tracing; returns (output, BassKernelResults)."""
    nc = _get_program()
    in_maps = _prep_inputs(inputs)
    res = run_bass_kernel_spmd(nc, in_maps, core_ids=list(range(NCORES)),
                               trace=True, trace_cores=list(range(NCORES)))
    return _unshard_output(res.results), res
